# revision 7
# baseline (speedup 1.0000x reference)
"""DecoderRnn single-step kernel for 8 Trainium2 NeuronCores (Bass/Tile).

Math (matches the PyTorch-style reference):
    x = relu(emb[token])                       # [1, 1024]
    for l in 0..1:  h_l = GRUCell(x, hidden[l]); x = h_l
    logits = x @ w_out.T + b_out               # [1, 50257]
    out = (log_softmax(logits), stack(h_0, h_1))

Sharding (8 cores):
  - emb rows, w_out rows (vocab dim), b_out sharded 8 ways (6400 rows/core,
    vocab padded 50257 -> 51200, pad bias = -1e30 so pad never matters).
  - GRU gate rows sharded 8 ways -> each core computes 128 hidden units per
    layer; an AllGather rebuilds the full hidden vector.
  - Embedding row: each core holds its emb shard plus zero guard rows; the
    owning core gathers the real row via a dynamic-offset DMA, other cores
    gather zeros; AllReduce(add) broadcasts the row.
  - log-softmax: per-core (max, sumexp) stats are AllGathered (32 B/core) and
    the global correction is applied locally; each core writes its shard.

Compute dtype: fp16 matmul operands (host pre-casts weights), fp32 PSUM
accumulation and fp32 elementwise math.
"""

import sys
import os

for _p in ("/opt/trn_rl_repo", "/root/.axon_site/_ro/trn_rl_repo"):
    if os.path.isdir(_p) and _p not in sys.path:
        sys.path.insert(0, _p)

import numpy as np

import concourse.bass as bass
import concourse.mybir as mybir
import concourse.tile as tile
import concourse.bacc as bacc
from concourse.bass_utils import run_bass_kernel_spmd

F16 = mybir.dt.float16
F32 = mybir.dt.float32
I32 = mybir.dt.int32

V = 50257          # vocab
H = 1024           # hidden size
L = 2              # layers
NCORES = 8
VS = 6400          # vocab shard per core (8 * 6400 = 51200 >= V)
HT = H // 128      # 8 h-tiles
G3 = 3 * 128       # 384 gate rows per core per h-tile
NEG_BIG = -1.0e30

# w_out stream blocks (vocab cols per DMA), and <=512-wide psum chunks
BLOCK_W = [1024] * 6 + [256]                 # sum = 6400
CHUNKS = []                                  # (block, off_in_block, width, vocab_off)
_off = 0
for _b, _bw in enumerate(BLOCK_W):
    _j = 0
    while _j < _bw:
        _cw = min(512, _bw - _j)
        CHUNKS.append((_b, _j, _cw, _off))
        _off += _cw
        _j += _cw
NCH = len(CHUNKS)                            # 13

TRACE = False
LAST_RESULT = None
_STATE = {}


def _build_nc(do_compile=True):
    nc = bacc.Bacc("TRN2", target_bir_lowering=False, debug=False,
                   num_devices=NCORES)

    tok_d = nc.dram_tensor("tok", [1, 1], I32, kind="ExternalInput")
    base_d = nc.dram_tensor("base2", [1, 1], I32, kind="ExternalInput")
    emb_d = nc.dram_tensor("emb_ext", [VS + 2, H], F32, kind="ExternalInput")
    hidf_d = nc.dram_tensor("hid16", [L, 128, HT], F16, kind="ExternalInput")
    hido_d = nc.dram_tensor("hid_own", [L, 1, 128], F32, kind="ExternalInput")
    wih_d = nc.dram_tensor("wih_t", [L, 128, HT * G3], F16, kind="ExternalInput")
    whh_d = nc.dram_tensor("whh_t", [L, 128, HT * G3], F16, kind="ExternalInput")
    gb_d = nc.dram_tensor("gbias", [L, 1, 512], F32, kind="ExternalInput")
    wout_d = nc.dram_tensor("wout_t", [128, HT * VS], F16, kind="ExternalInput")
    bout_d = nc.dram_tensor("bout", [1, VS], F32, kind="ExternalInput")
    id8_d = nc.dram_tensor("id8", [8, 8], F16, kind="ExternalInput")

    lg_d = nc.dram_tensor("out_logits", [1, VS], F32, kind="ExternalOutput")
    oh_d = nc.dram_tensor("out_hidden", [L, H], F32, kind="ExternalOutput")

    rg = [list(range(NCORES))]

    with tile.TileContext(nc) as tc:
        with (
            tc.tile_pool(name="small", bufs=1) as sp,
            tc.tile_pool(name="gru", bufs=1) as gp,
            tc.tile_pool(name="wstream", bufs=4) as wp,
            tc.tile_pool(name="psg", bufs=1, space="PSUM") as psg,
            tc.tile_pool(name="pslg", bufs=4, space="PSUM") as pslg,
            tc.tile_pool(name="psx", bufs=2, space="PSUM") as psx,
            tc.tile_pool(name="dram", bufs=1, space="DRAM") as dr,
        ):
            # ---------------- static loads (sync HWDGE ring) ----------------
            tok_sb = sp.tile([1, 1], I32)
            nc.sync.dma_start(tok_sb[:], tok_d[:])
            base_sb = sp.tile([1, 1], I32)
            nc.sync.dma_start(base_sb[:], base_d[:])
            id8_sb = sp.tile([8, 8], F16)
            nc.sync.dma_start(id8_sb[:], id8_d[:])
            hidf_sb = [gp.tile([128, HT], F16, name=f"hidf{l}") for l in range(L)]
            hido_sb = [sp.tile([1, 128], F32, name=f"hido{l}") for l in range(L)]
            gb_sb = [sp.tile([1, 512], F32, name=f"gb{l}") for l in range(L)]
            for l in range(L):
                nc.sync.dma_start(hidf_sb[l][:], hidf_d[l])
                nc.sync.dma_start(hido_sb[l][:], hido_d[l])
                nc.sync.dma_start(gb_sb[l][:], gb_d[l])
            bout_sb = sp.tile([1, VS], F32)
            nc.sync.dma_start(bout_sb[:], bout_d[:])
            wih_sb = [gp.tile([128, HT * G3], F16, name=f"wih{l}") for l in range(L)]
            whh_sb = [gp.tile([128, HT * G3], F16, name=f"whh{l}") for l in range(L)]
            for l in range(L):
                nc.sync.dma_start(whh_sb[l][:], whh_d[l])
                nc.sync.dma_start(wih_sb[l][:], wih_d[l])
            # big w_out stream blocks, last on the sync ring
            wv_sb = []
            woff = 0
            for b, bw in enumerate(BLOCK_W):
                wv = wp.tile([128, HT * bw], F16, tag="wv", name=f"wv{b}",
                             padded_shape=[128, HT * BLOCK_W[0]])
                nc.sync.dma_start(wv[:], wout_d[:, woff:woff + HT * bw])
                wv_sb.append(wv)
                woff += HT * bw

            # ---------------- embedding gather + AllReduce ----------------
            # local = tok - (c*VS - 1) clamped to [0, VS+1]; emb_ext rows 0 and
            # VS+1 are zero guards, rows 1..VS are this core's shard.
            i1 = sp.tile([1, 1], I32)
            nc.vector.tensor_tensor(i1[:], tok_sb[:], base_sb[:],
                                    mybir.AluOpType.subtract)
            i2 = sp.tile([1, 1], I32)
            nc.vector.tensor_scalar_max(i2[:], i1[:], 0)
            i3 = sp.tile([1, 1], I32)
            nc.vector.tensor_scalar_min(i3[:], i2[:], VS + 1)
            iv = nc.values_load(i3[0:1, 0:1], engines=(mybir.EngineType.Pool,),
                                min_val=0, max_val=VS + 1,
                                skip_runtime_bounds_check=True)
            row_sb = sp.tile([1, H], F32)
            nc.gpsimd.dma_start(row_sb[:], emb_d.ap()[bass.ds(iv, 1), :])

            cc_x_in = dr.tile([1, H], F32)
            cc_x_out = dr.tile([1, H], F32, addr_space="Shared")
            nc.scalar.dma_start(cc_x_in[:], row_sb[:])
            nc.gpsimd.collective_compute(
                "AllReduce", mybir.AluOpType.add, replica_groups=rg,
                ins=[cc_x_in[:]], outs=[cc_x_out[:]])

            def vec_to_cols(cc_view, relu, name):
                """[8,128] f32 DRAM view -> fp16 [128, 8] x-columns in SBUF."""
                v8f = sp.tile([8, 128], F32, name=f"{name}_f")
                nc.scalar.dma_start(v8f[:], cc_view)
                v8h = sp.tile([8, 128], F16, name=f"{name}_h")
                nc.vector.tensor_copy(v8h[:], v8f[:])
                ps = psx.tile([128, 8], F16, tag="xps", name=f"{name}_ps")
                nc.tensor.matmul(ps[:], v8h[:], id8_sb[:], is_transpose=True)
                out = sp.tile([128, HT], F16, name=f"{name}_x")
                if relu:
                    nc.scalar.activation(out[:], ps[:],
                                         mybir.ActivationFunctionType.Relu)
                else:
                    nc.scalar.copy(out[:], ps[:])
                return out

            x1_sb = vec_to_cols(cc_x_out.rearrange("a (c p) -> (a c) p", c=8),
                                relu=True, name="x1")

            # ---------------- GRU layers ----------------
            def gru_layer(l, x_sb):
                pa = psg.tile([1, G3], F32, tag="pa", name=f"pa{l}")
                pb = psg.tile([1, 128], F32, tag="pb", name=f"pb{l}")
                # h-side first: host hidden is available early
                for h in range(HT):
                    nc.tensor.matmul(pa[0:1, 0:256], hidf_sb[l][:, h:h + 1],
                                     whh_sb[l][:, h * G3: h * G3 + 256],
                                     start=(h == 0), stop=False)
                for h in range(HT):
                    nc.tensor.matmul(pb[0:1, 0:128], hidf_sb[l][:, h:h + 1],
                                     whh_sb[l][:, h * G3 + 256: h * G3 + 384],
                                     start=(h == 0), stop=(h == HT - 1))
                # x-side
                for h in range(HT):
                    nc.tensor.matmul(pa[0:1, 0:256], x_sb[:, h:h + 1],
                                     wih_sb[l][:, h * G3: h * G3 + 256],
                                     start=False, stop=(h == HT - 1))
                for h in range(HT):
                    nc.tensor.matmul(pa[0:1, 256:384], x_sb[:, h:h + 1],
                                     wih_sb[l][:, h * G3 + 256: h * G3 + 384],
                                     start=(h == 0), stop=(h == HT - 1))
                # gate math on partition 0, f32
                rz_pre = sp.tile([1, 256], F32, name=f"rzp{l}")
                nc.vector.tensor_tensor(rz_pre[:], pa[0:1, 0:256],
                                        gb_sb[l][0:1, 0:256], mybir.AluOpType.add)
                rz = sp.tile([1, 256], F32, name=f"rz{l}")
                nc.scalar.activation(rz[:], rz_pre[:],
                                     mybir.ActivationFunctionType.Sigmoid)
                hn = sp.tile([1, 128], F32, name=f"hn{l}")
                nc.vector.tensor_tensor(hn[:], pb[0:1, 0:128],
                                        gb_sb[l][0:1, 384:512], mybir.AluOpType.add)
                rhn = sp.tile([1, 128], F32, name=f"rhn{l}")
                nc.vector.tensor_tensor(rhn[:], rz[0:1, 0:128], hn[:],
                                        mybir.AluOpType.mult)
                npre = sp.tile([1, 128], F32, name=f"npre{l}")
                nc.vector.tensor_tensor(npre[:], pa[0:1, 256:384], rhn[:],
                                        mybir.AluOpType.add)
                npre2 = sp.tile([1, 128], F32, name=f"npre2{l}")
                nc.vector.tensor_tensor(npre2[:], npre[:],
                                        gb_sb[l][0:1, 256:384], mybir.AluOpType.add)
                nt = sp.tile([1, 128], F32, name=f"nt{l}")
                nc.scalar.activation(nt[:], npre2[:],
                                     mybir.ActivationFunctionType.Tanh)
                d = sp.tile([1, 128], F32, name=f"d{l}")
                nc.vector.tensor_tensor(d[:], hido_sb[l][:], nt[:],
                                        mybir.AluOpType.subtract)
                zd = sp.tile([1, 128], F32, name=f"zd{l}")
                nc.vector.tensor_tensor(zd[:], rz[0:1, 128:256], d[:],
                                        mybir.AluOpType.mult)
                hnew = sp.tile([1, 128], F32, name=f"hnew{l}")
                nc.vector.tensor_tensor(hnew[:], nt[:], zd[:], mybir.AluOpType.add)
                # AllGather slice -> full hidden vector; also emit out_hidden[l]
                cc_in = dr.tile([1, 128], F32, name=f"cch_in{l}")
                cc_out = dr.tile([8, 128], F32, addr_space="Shared",
                                 name=f"cch_out{l}")
                nc.scalar.dma_start(cc_in[:], hnew[:])
                nc.gpsimd.collective_compute(
                    "AllGather", mybir.AluOpType.bypass, replica_groups=rg,
                    ins=[cc_in[:]], outs=[cc_out[:]])
                nc.scalar.dma_start(oh_d.ap()[l].rearrange("(a b) -> a b", a=8),
                                    cc_out[:])
                return cc_out

            cc_h1 = gru_layer(0, x1_sb)
            x2_sb = vec_to_cols(cc_h1[:], relu=False, name="x2")
            cc_h2 = gru_layer(1, x2_sb)
            xp_sb = vec_to_cols(cc_h2[:], relu=False, name="xp")

            # ---------------- logits + per-chunk stats ----------------
            logits_sb = sp.tile([1, VS], F32)
            smax = sp.tile([1, NCH], F32)
            nsmax = sp.tile([1, NCH], F32)
            ssum = sp.tile([1, NCH], F32)
            for ci, (b, j, cw, voff) in enumerate(CHUNKS):
                pc = pslg.tile([1, 512], F32, tag="lg", name=f"lg{ci}")
                for h in range(HT):
                    bw = BLOCK_W[b]
                    nc.tensor.matmul(pc[0:1, 0:cw], xp_sb[:, h:h + 1],
                                     wv_sb[b][:, h * bw + j: h * bw + j + cw],
                                     start=(h == 0), stop=(h == HT - 1))
                lsl = logits_sb[0:1, voff:voff + cw]
                nc.vector.tensor_tensor(lsl, pc[0:1, 0:cw],
                                        bout_sb[0:1, voff:voff + cw],
                                        mybir.AluOpType.add)
                nc.vector.tensor_reduce(smax[0:1, ci:ci + 1], lsl,
                                        mybir.AxisListType.X, mybir.AluOpType.max)
                nc.vector.tensor_scalar_mul(nsmax[0:1, ci:ci + 1],
                                            smax[0:1, ci:ci + 1], -1.0)
                esc = sp.tile([1, 512], F32, tag="esc", name=f"esc{ci}", bufs=2)
                nc.scalar.activation(esc[0:1, 0:cw], lsl,
                                     mybir.ActivationFunctionType.Exp,
                                     bias=nsmax[0:1, ci:ci + 1],
                                     accum_out=ssum[0:1, ci:ci + 1])

            # ---------------- local -> global logsumexp ----------------
            lmax = sp.tile([1, 1], F32)
            nc.vector.tensor_reduce(lmax[:], smax[:], mybir.AxisListType.X,
                                    mybir.AluOpType.max)
            nlmax = sp.tile([1, 1], F32)
            nc.vector.tensor_scalar_mul(nlmax[:], lmax[:], -1.0)
            ec = sp.tile([1, NCH], F32)
            nc.scalar.activation(ec[:], smax[:],
                                 mybir.ActivationFunctionType.Exp,
                                 bias=nlmax[0:1, 0:1])
            ws = sp.tile([1, NCH], F32)
            nc.vector.tensor_tensor(ws[:], ec[:], ssum[:], mybir.AluOpType.mult)
            lsum = sp.tile([1, 1], F32)
            nc.vector.tensor_reduce(lsum[:], ws[:], mybir.AxisListType.X,
                                    mybir.AluOpType.add)
            st = sp.tile([1, 8], F32)
            nc.vector.memset(st[:], 0.0)
            nc.vector.tensor_copy(st[0:1, 0:1], lmax[:])
            nc.vector.tensor_copy(st[0:1, 1:2], lsum[:])

            cc_s_in = dr.tile([1, 8], F32)
            cc_s_out = dr.tile([1, 64], F32, addr_space="Shared")
            nc.scalar.dma_start(cc_s_in[:], st[:])
            nc.gpsimd.collective_compute(
                "AllGather", mybir.AluOpType.bypass, replica_groups=rg,
                ins=[cc_s_in[:]], outs=[cc_s_out[:]])
            gst = sp.tile([1, 64], F32)
            nc.scalar.dma_start(gst[:], cc_s_out[:])
            gst3 = gst.rearrange("a (c s) -> a c s", s=8)
            cmax = sp.tile([1, 8], F32)
            nc.vector.tensor_copy(cmax.rearrange("a (c s) -> a c s", s=1),
                                  gst3[:, :, 0:1])
            csum = sp.tile([1, 8], F32)
            nc.vector.tensor_copy(csum.rearrange("a (c s) -> a c s", s=1),
                                  gst3[:, :, 1:2])
            gmax = sp.tile([1, 1], F32)
            nc.vector.tensor_reduce(gmax[:], cmax[:], mybir.AxisListType.X,
                                    mybir.AluOpType.max)
            ngmax = sp.tile([1, 1], F32)
            nc.vector.tensor_scalar_mul(ngmax[:], gmax[:], -1.0)
            gec = sp.tile([1, 8], F32)
            nc.scalar.activation(gec[:], cmax[:],
                                 mybir.ActivationFunctionType.Exp,
                                 bias=ngmax[0:1, 0:1])
            gws = sp.tile([1, 8], F32)
            nc.vector.tensor_tensor(gws[:], gec[:], csum[:], mybir.AluOpType.mult)
            gsum = sp.tile([1, 1], F32)
            nc.vector.tensor_reduce(gsum[:], gws[:], mybir.AxisListType.X,
                                    mybir.AluOpType.add)
            gln = sp.tile([1, 1], F32)
            nc.scalar.activation(gln[:], gsum[:],
                                 mybir.ActivationFunctionType.Ln)
            lse = sp.tile([1, 1], F32)
            nc.vector.tensor_tensor(lse[:], gln[:], gmax[:], mybir.AluOpType.add)
            nlse = sp.tile([1, 1], F32)
            nc.vector.tensor_scalar_mul(nlse[:], lse[:], -1.0)

            # ---------------- logprobs = logits - lse; write out ----------------
            lp = sp.tile([1, VS], F32)
            for ci, (b, j, cw, voff) in enumerate(CHUNKS):
                src = logits_sb[0:1, voff:voff + cw]
                dst = lp[0:1, voff:voff + cw]
                if ci % 2 == 0:
                    nc.scalar.activation(dst, src,
                                         mybir.ActivationFunctionType.Identity,
                                         bias=nlse[0:1, 0:1])
                else:
                    nc.vector.tensor_scalar(dst, src, nlse[0:1, 0:1], None,
                                            mybir.AluOpType.add)
            nc.sync.dma_start(lg_d[:], lp[:])

    if do_compile:
        nc.compile()
    return nc


def _get_nc():
    if "nc" not in _STATE:
        _STATE["nc"] = _build_nc()
    return _STATE["nc"]


def _prep_inputs(input, hidden, emb, w_ih, w_hh, b_ih, b_hh, w_out, b_out):
    """Host-side sharding/layout prep. Returns per-core input maps."""
    tok = int(np.asarray(input).reshape(-1)[0])
    hidden = np.asarray(hidden, dtype=np.float32).reshape(L, H)
    emb = np.ascontiguousarray(np.asarray(emb, dtype=np.float32))
    w_ih = np.asarray(w_ih, dtype=np.float32)
    w_hh = np.asarray(w_hh, dtype=np.float32)
    b_ih = np.asarray(b_ih, dtype=np.float32)
    b_hh = np.asarray(b_hh, dtype=np.float32)
    w_out = np.asarray(w_out, dtype=np.float32)
    b_out = np.asarray(b_out, dtype=np.float32)

    tok_arr = np.array([[tok]], dtype=np.int32)
    id8 = np.eye(8, dtype=np.float16)

    # replicated hidden, fp16 column layout [L, 128, 8]
    hid16 = np.ascontiguousarray(
        hidden.reshape(L, HT, 128).transpose(0, 2, 1)).astype(np.float16)

    in_maps = []
    for c in range(NCORES):
        lo = c * VS
        hi = min(V, lo + VS)
        emb_ext = np.zeros((VS + 2, H), dtype=np.float32)
        if hi > lo:
            emb_ext[1:1 + hi - lo] = emb[lo:hi]

        hid_own = np.ascontiguousarray(
            hidden[:, c * 128:(c + 1) * 128].reshape(L, 1, 128))

        # GRU weight slices: [L, 128p, h*384 + g*128 + m]
        wih_t = np.empty((L, 128, HT * G3), dtype=np.float16)
        whh_t = np.empty((L, 128, HT * G3), dtype=np.float16)
        for l in range(L):
            for (w, dstbuf) in ((w_ih, wih_t), (w_hh, whh_t)):
                # wsl[g*128+m, :] = w[l, g*H + c*128 + m, :]
                wsl = np.concatenate(
                    [w[l, g * H + c * 128: g * H + (c + 1) * 128, :]
                     for g in range(3)], axis=0)            # [384, 1024]
                # dst[p, h*384 + gm] = wsl[gm, h*128 + p]
                r = wsl.T.reshape(HT, 128, G3)              # [h, p, gm]
                dstbuf[l] = np.ascontiguousarray(
                    r.transpose(1, 0, 2).reshape(128, HT * G3)).astype(np.float16)

        gbias = np.empty((L, 1, 512), dtype=np.float32)
        for l in range(L):
            sl = slice(c * 128, (c + 1) * 128)
            br = b_ih[l, 0 * H:1 * H][sl] + b_hh[l, 0 * H:1 * H][sl]
            bz = b_ih[l, 1 * H:2 * H][sl] + b_hh[l, 1 * H:2 * H][sl]
            bin_ = b_ih[l, 2 * H:3 * H][sl]
            bhn = b_hh[l, 2 * H:3 * H][sl]
            gbias[l, 0] = np.concatenate([br, bz, bin_, bhn])

        # w_out shard, transposed + blocked:
        # wout_t[p, blockoff + h*bw + jj] = w_out_pad[c*VS + vb*1024 + jj, h*128+p]
        wc = np.zeros((VS, H), dtype=np.float32)
        if hi > lo:
            wc[: hi - lo] = w_out[lo:hi]
        r = wc.T.reshape(HT, 128, VS)                       # [h, p, v]
        r = r.transpose(1, 0, 2)                            # [p, h, v]
        parts = []
        vo = 0
        for bw in BLOCK_W:
            parts.append(r[:, :, vo:vo + bw].reshape(128, HT * bw))
            vo += bw
        wout_t = np.ascontiguousarray(
            np.concatenate(parts, axis=1)).astype(np.float16)

        bout = np.full((1, VS), NEG_BIG, dtype=np.float32)
        if hi > lo:
            bout[0, : hi - lo] = b_out[lo:hi]

        in_maps.append({
            "tok": tok_arr,
            "base2": np.array([[c * VS - 1]], dtype=np.int32),
            "emb_ext": emb_ext,
            "hid16": hid16,
            "hid_own": hid_own,
            "wih_t": wih_t,
            "whh_t": whh_t,
            "gbias": gbias,
            "wout_t": wout_t,
            "bout": bout,
            "id8": id8,
        })
    return in_maps


def kernel(input, hidden, emb, w_ih, w_hh, b_ih, b_hh, w_out, b_out):
    global LAST_RESULT
    nc = _get_nc()
    in_maps = _prep_inputs(input, hidden, emb, w_ih, w_hh, b_ih, b_hh,
                           w_out, b_out)
    res = run_bass_kernel_spmd(nc, in_maps, core_ids=list(range(NCORES)),
                               trace=TRACE)
    LAST_RESULT = res
    logprobs = np.concatenate(
        [res.results[c]["out_logits"][0] for c in range(NCORES)])[:V]
    logprobs = np.ascontiguousarray(logprobs.reshape(1, V), dtype=np.float32)
    hidden_out = np.ascontiguousarray(
        res.results[0]["out_hidden"].reshape(L, 1, H), dtype=np.float32)
    return logprobs, hidden_out
